# revision 32
# baseline (speedup 1.0000x reference)
"""Distributional Q-network (C51) Trainium2 kernel — 8-core data parallel.

Algorithm per 512-row tile (feature-major fp16 MLP):
  cat(obs,act) -> 3x(Linear+ReLU) -> Linear -> exp (unnormalized softmax)
  b = clamp(u + c*j, 2, 102); f = round(b-0.5); phi = b - f
  prefix-sum matmuls (ltri) transpose e / e*phi to batch-major P,Q
  scatter indices are computed BY THE PE: idx = f + MBIG*(f_next - f)
  - MBIG via a banded matrix matmul plus a K=1 constant-injection matmul
  (f monotone with steps in {0,1} since c<1, so valid last-of-run indices
  survive, invalid ones go negative and local_scatter ignores them).
  gpsimd local_scatter places prefix values at last-of-run bins; tail
  bins k > f[100] are filled via an iota compare (f100 rides along in the
  idx pad slot as f100-126, still negative). Shifted differences then
  produce proj, normalized at the very end by 1/total (normalize-late:
  raw numerator magnitudes are ~100, safe in fp16).

Output rows are assigned to lanes via a host-side input-column
permutation so each partition writes one contiguous DRAM block
(row = p*128 + t*NSUB + s), giving >=3KB output DMA runs. Output is
fp16 on device, converted to fp32 on host.
"""
import numpy as np
from contextlib import ExitStack

BATCH = 131072
NCORES = 8
RPC = BATCH // NCORES          # rows per core
BT = 512                       # rows per tile (4 subtiles of 128)
NSUB = BT // 128
A = 101                        # atoms
NE = 212                       # scatter elems: A[0:106) B[106:212)
NI = 102                       # scatter idxs per call (f100 pad at col 101);
                               # the B region reuses the same idxs with the
                               # +106 offset coming from the output AP base
MBIG = 256.0                   # invalid-index offset; must stay negative even
                               # after the +106 B-region shift (f16-exact)
JG = 16                        # rows-per-partition per group (4 tiles)
GT = JG // NSUB                # tiles per group

_CACHE = {}


def build_program(rpc=RPC, n_repeat=1):
    import concourse.bacc as bacc
    import concourse.bass as bass
    import concourse.mybir as mybir
    import concourse.tile as tile

    f16, f32, i16 = mybir.dt.float16, mybir.dt.float32, mybir.dt.int16
    nt = rpc // BT
    ngrp = nt // GT
    gcols = GT * BT            # input columns per group

    nc = bacc.Bacc(None, target_bir_lowering=False)
    xT = nc.declare_dram_parameter("xT", [80, rpc], f16, isOutput=False)
    uc4 = nc.declare_dram_parameter("uc4", [4, rpc], f16, isOutput=False)
    w1a = nc.declare_dram_parameter("w1a", [80, 128], f16, isOutput=False)
    w1b = nc.declare_dram_parameter("w1b", [80, 128], f16, isOutput=False)
    w2a = nc.declare_dram_parameter("w2a", [128, 128], f16, isOutput=False)
    w2b = nc.declare_dram_parameter("w2b", [128, 128], f16, isOutput=False)
    w3 = nc.declare_dram_parameter("w3", [128, 64], f16, isOutput=False)
    w4 = nc.declare_dram_parameter("w4", [64, A], f16, isOutput=False)
    ltri = nc.declare_dram_parameter("ltri", [A, 102], f16, isOutput=False)
    mmat = nc.declare_dram_parameter("mmat", [A, 102], f16, isOutput=False)
    crow = nc.declare_dram_parameter("crow", [1, 102], f16, isOutput=False)
    m4 = nc.declare_dram_parameter("m4", [4, A], f16, isOutput=False)
    bia = nc.declare_dram_parameter("bia", [128, 5], f32, isOutput=False)
    out = nc.declare_dram_parameter("out", [rpc, A], f16, isOutput=True)
    outv = out.rearrange("(p j) a -> p j a", p=128)
    sout = nc.declare_dram_parameter("sout", [rpc, 2], f16, isOutput=True)
    soutv = sout.rearrange("(p j) a -> p j a", p=128)

    es = ExitStack()
    with tile.TileContext(nc) as tc:
        wp = es.enter_context(tc.tile_pool(name="wp", bufs=1))
        io = es.enter_context(tc.tile_pool(name="io", bufs=2))
        mid = es.enter_context(tc.tile_pool(name="mid", bufs=3))
        grp = es.enter_context(tc.tile_pool(name="grp", bufs=2))
        ps = es.enter_context(
            tc.tile_pool(name="ps", bufs=1, space=bass.MemorySpace.PSUM))

        # ---- load weights/consts once ----
        wt = {}
        for h, t in [(w1a, "w1a"), (w1b, "w1b"), (w2a, "w2a"), (w2b, "w2b"),
                     (w3, "w3"), (w4, "w4"), (ltri, "ltri"), (mmat, "mmat"),
                     (crow, "crow"), (m4, "m4")]:
            wt[t] = wp.tile(h.shape, f16, tag=t, name=t)
            nc.sync.dma_start(wt[t][:], h[:])
        bt = wp.tile([128, 5], f32, tag="bia")
        nc.sync.dma_start(bt[:], bia[:])
        b1a, b1b = bt[:, 0:1], bt[:, 1:2]
        b2, b3, b4 = bt[:, 2:3], bt[0:64, 3:4], bt[0:A, 4:5]

        ones1 = wp.tile([1, 128], f16, tag="ones1")
        nc.vector.memset(ones1[:], 1.0)

        Relu = mybir.ActivationFunctionType.Relu
        Exp = mybir.ActivationFunctionType.Exp
        Copy = mybir.ActivationFunctionType.Copy
        op = mybir.AluOpType

        for g in [gg for _ in range(n_repeat) for gg in range(ngrp)]:
            xg = io.tile([80, gcols], f16, tag="xg")
            nc.sync.dma_start(xg[:], xT[:, g * gcols:(g + 1) * gcols])
            ug = io.tile([4, gcols], f16, tag="ug")
            nc.sync.dma_start(ug[:], uc4[:, g * gcols:(g + 1) * gcols])

            data = grp.tile([128, JG, 204], f16, tag="data")
            idx = grp.tile([128, JG, NI], i16, tag="idx")
            dst = grp.tile([128, JG, NE], f16, tag="dst")
            sog = grp.tile([128, JG, 2], f16, tag="sog")

            for tt in range(GT):
                xs = slice(tt * BT, (tt + 1) * BT)

                # ---- MLP (feature-major) ----
                h1a_ps = ps.tile([128, BT], f32, tag="h1a")
                nc.tensor.matmul(h1a_ps[:], wt["w1a"][:], xg[:, xs])
                h1b_ps = ps.tile([128, BT], f32, tag="h1b")
                nc.tensor.matmul(h1b_ps[:], wt["w1b"][:], xg[:, xs])
                h1a = mid.tile([128, BT], f16, tag="h1a")
                nc.scalar.activation(h1a[:], h1a_ps[:], Relu, bias=b1a)
                h1b = mid.tile([128, BT], f16, tag="h1b")
                nc.scalar.activation(h1b[:], h1b_ps[:], Relu, bias=b1b)

                h2_ps = ps.tile([128, BT], f32, tag="h2")
                nc.tensor.matmul(h2_ps[:], wt["w2a"][:], h1a[:],
                                 start=True, stop=False)
                nc.tensor.matmul(h2_ps[:], wt["w2b"][:], h1b[:],
                                 start=False, stop=True)
                h2 = mid.tile([128, BT], f16, tag="h2")
                nc.scalar.activation(h2[:], h2_ps[:], Relu, bias=b2)

                h3_ps = ps.tile([64, BT], f32, tag="h3lg")
                nc.tensor.matmul(h3_ps[:], wt["w3"][:], h2[:])
                h3 = mid.tile([64, BT], f16, tag="h3")
                nc.scalar.activation(h3[:], h3_ps[:], Relu, bias=b3)

                lg_ps = ps.tile([A, BT], f32, tag="h3lg")
                nc.tensor.matmul(lg_ps[:], wt["w4"][:], h3[:])
                e = mid.tile([A, BT], f16, tag="e")
                nc.scalar.activation(e[:], lg_ps[:], Exp, bias=b4)

                # ---- b = clamp(u + c*j, 2, 102), f = round(b-0.5), phi ----
                b_ps = ps.tile([A, BT], f32, tag="b")
                nc.tensor.matmul(b_ps[:], wt["m4"][:], ug[:, xs])
                bcl = mid.tile([A, BT], f32, tag="bcl")
                nc.vector.tensor_scalar(bcl[:], b_ps[:], 102.0, 2.0,
                                        op.min, op.max)
                # f = round(b-0.5) in one op; tie-to-even giving (f-1, phi=1)
                # is exactly compensated by the hist_dphi[k-1] combine term
                ffm = mid.tile([A, BT], f16, tag="ffm")
                nc.vector.tensor_scalar(ffm[:], bcl[:], 8388607.5, 8388608.0,
                                        op.add, op.subtract)
                phi = mid.tile([A, BT], f16, tag="phi")
                nc.vector.tensor_tensor(phi[:], bcl[:], ffm[:], op.subtract)
                ephi = mid.tile([A, BT], f16, tag="ephi")
                nc.vector.tensor_tensor(ephi[:], e[:], phi[:], op.mult)

                # ---- transpose to batch-major via PE ----
                # PQ: bank 0 = prefix(e), bank 1 = prefix(e*phi), cols incl total
                PQ_ps = ps.tile([128, 2, NSUB, 128], f32, tag="PQ")
                X_ps = ps.tile([128, NSUB, 102], f32, tag="X")
                for s in range(NSUB):
                    sl = slice(s * 128, (s + 1) * 128)
                    nc.tensor.matmul(PQ_ps[:, 0, s, 0:102], e[:, sl],
                                     wt["ltri"][:])
                    nc.tensor.matmul(PQ_ps[:, 1, s, 0:102], ephi[:, sl],
                                     wt["ltri"][:])
                    # idx = f + MBIG*(f_next - f) - MBIG; f100-126 in col 101
                    nc.tensor.matmul(X_ps[:, s, :], ffm[:, sl],
                                     wt["mmat"][:], start=True, stop=False)
                    nc.tensor.matmul(X_ps[:, s, :], ones1[:], wt["crow"][:],
                                     start=False, stop=True)

                j4 = slice(tt * NSUB, (tt + 1) * NSUB)
                # scatter payload: [P(102) | Q(102)] per row, raw scale
                nc.scalar.activation(
                    data[:, j4, 0:204].rearrange("p j (r x) -> p j r x", r=2),
                    PQ_ps[:, :, :, 0:102].transpose([0, 2, 1, 3]), Copy)
                nc.scalar.activation(idx[:, j4, 0:102], X_ps[:], Copy)
                nc.vector.tensor_scalar(sog[:, j4, 1:2], X_ps[:, :, 101:102],
                                        126.0, None, op.add)

                for s in range(NSUB):
                    j = tt * NSUB + s
                    nc.gpsimd.local_scatter(dst[:, j, 0:106],
                                            data[:, j, 0:102],
                                            idx[:, j, :], channels=128,
                                            num_elems=106, num_idxs=NI)
                    nc.gpsimd.local_scatter(dst[:, j, 106:212],
                                            data[:, j, 102:204],
                                            idx[:, j, :], channels=128,
                                            num_elems=106, num_idxs=NI)

            # ---- batched combine over the whole group ----
            # dst-only stencil; the 2-point tail correction
            # (1-qn) at f100-1 and qn at f100 is applied on the HOST from
            # the exported (qn, f100) side output.
            s2 = grp.tile([128, JG, A], f16, tag="s2")
            d1 = grp.tile([128, JG, A], f16, tag="d1")
            rec = grp.tile([128, JG, 1], f16, tag="rec")
            osb = grp.tile([128, JG, A], f16, tag="osb")

            nc.vector.tensor_tensor(d1[:], dst[:, :, 2:103],
                                    dst[:, :, 1:102], op.subtract)
            nc.vector.scalar_tensor_tensor(s2[:], dst[:, :, 107:208], 2.0,
                                           dst[:, :, 108:209], op.mult,
                                           op.subtract)
            nc.vector.tensor_tensor(s2[:], s2[:], dst[:, :, 106:207],
                                    op.subtract)
            nc.vector.tensor_tensor(d1[:], d1[:], s2[:], op.add)
            with nc.allow_low_precision(reason="f16 recip fine at 2e-2 tol"):
                nc.vector.reciprocal(rec[:], data[:, :, 101:102])
            nc.vector.tensor_tensor(osb[:], d1[:],
                                    rec[:].broadcast_to([128, JG, A]), op.mult)
            nc.vector.tensor_tensor(sog[:, :, 0:1], data[:, :, 203:204],
                                    rec[:], op.mult)
            nc.sync.dma_start(outv[:, g * JG:(g + 1) * JG, :], osb[:])
            nc.sync.dma_start(soutv[:, g * JG:(g + 1) * JG, :], sog[:])

        es.close()

    nc.compile()
    return nc


def _perm(rpc=RPC):
    nt = rpc // BT
    t = np.arange(nt)[:, None, None]
    s = np.arange(NSUB)[None, :, None]
    p = np.arange(128)[None, None, :]
    return (p * (nt * NSUB) + t * NSUB + s).reshape(-1)


def prep_inputs(obs, actions, rewards, bootstrap, discount,
                W1, b1, W2, b2, W3, b3, W4, b4):
    c = (bootstrap * discount).astype(np.float32)
    u = (5.0 * rewards - 50.0 * c + 52.0).astype(np.float32)
    u_hi = u.astype(np.float16)
    u_lo = (u - u_hi.astype(np.float32)).astype(np.float16)
    c_hi = c.astype(np.float16)
    c_lo = (c - c_hi.astype(np.float32)).astype(np.float16)
    uc4 = np.stack([u_hi, u_lo, c_hi, c_lo])                    # [4, B]
    xT = np.ascontiguousarray(
        np.concatenate([obs, actions], 1).T.astype(np.float16))  # [80, B]
    W4c = W4 - W4.mean(axis=1, keepdims=True)
    b4c = b4 - b4.mean()
    bia = np.zeros((128, 5), np.float32)
    bia[:, 0], bia[:, 1], bia[:, 2] = b1[:128], b1[128:], b2
    bia[:64, 3], bia[:A, 4] = b3, b4c
    jj = np.arange(A, dtype=np.float32)
    ltri = np.zeros((A, 102), np.float16)
    ltri[:, 101] = 1.0
    for m in range(A):
        ltri[:m + 1, m] = 1.0
    # idx matmul (206 cols): A-region idx = f + MBIG*(f_next-f) - MBIG,
    # B-region = A + 106, pads negative, C-region tail at f100+211/212.
    # valid idx = f itself (the +2 bin shift is already inside u/f)
    mmat = np.zeros((A, 102), np.float16)
    crow = np.zeros((1, 102), np.float16)
    for m in range(100):
        mmat[m, m] = 1.0 - MBIG
        mmat[m + 1, m] = MBIG
    crow[0, :100] = -MBIG
    mmat[100, 100] = 1.0
    mmat[100, 101] = 1.0
    crow[0, 101] = -126.0
    consts = {
        "w1a": W1[:, :128].astype(np.float16),
        "w1b": W1[:, 128:].astype(np.float16),
        "w2a": W2[:128].astype(np.float16),
        "w2b": W2[128:].astype(np.float16),
        "w3": W3.astype(np.float16),
        "w4": W4c.astype(np.float16),
        "ltri": ltri,
        "mmat": mmat,
        "crow": crow,
        "m4": np.stack([np.ones(A), np.ones(A), jj, jj]).astype(np.float16),
        "bia": bia,
    }
    return xT, uc4, consts


def kernel(obs, actions, rewards, bootstrap, discount, q_support,
           W1, b1, W2, b2, W3, b3, W4, b4):
    from concourse.bass_utils import run_bass_kernel_spmd
    if "nc" not in _CACHE:
        _CACHE["nc"] = build_program()
        _CACHE["perm"] = _perm()
    nc = _CACHE["nc"]
    perm = _CACHE["perm"]
    xT, uc4, consts = prep_inputs(obs, actions, rewards, bootstrap, discount,
                                  W1, b1, W2, b2, W3, b3, W4, b4)
    in_maps = []
    for i in range(NCORES):
        sl = slice(i * RPC, (i + 1) * RPC)
        m = {"xT": np.ascontiguousarray(xT[:, sl][:, perm]),
             "uc4": np.ascontiguousarray(uc4[:, sl][:, perm])}
        m.update(consts)
        in_maps.append(m)
    res = run_bass_kernel_spmd(nc, in_maps, list(range(NCORES))).results
    outf = np.concatenate([res[i]["out"].astype(np.float32)
                           for i in range(NCORES)], 0)
    so = np.concatenate([res[i]["sout"] for i in range(NCORES)], 0)
    # 2-point tail correction: += (1-qn) at f100-1 and qn at f100
    qn = so[:, 0].astype(np.float32)
    f100 = np.rint(so[:, 1]).astype(np.int64)
    rows = np.arange(outf.shape[0])
    k1 = f100 - 1
    m1 = k1 <= 100
    outf[rows[m1], k1[m1]] += 1.0 - qn[m1]
    m2 = f100 <= 100
    outf[rows[m2], f100[m2]] += qn[m2]
    return outf
